# revision 2
# baseline (speedup 1.0000x reference)
"""VQ codebook lookup kernel for Trainium2 (8 NeuronCores, data-parallel).

Computes out[b] = values[argmin_k ||x[b] - keys[k]||] for
x [65536, 512], keys/values [1024, 512] fp32.

Strategy (per core, batch shard of 8192 rows):
  - argmin distance == argmax s = 2*x.k - |k|^2 (sqrt and the |x|^2 row
    offset do not change the argmin).
  - Per 128-row tile, s[128, 1024] accumulates in PSUM from:
      1. ScalarE pre-writes the exact f32 -|k|^2 bias into the PSUM tile;
         all matmuls use start=False and accumulate onto it (warmup
         matmuls pre-set the has_written bits of the rotating PSUM
         buffers once, so the PE accumulates instead of overwriting).
      2. 8x fp32r matmuls (K=128, N=512): fp22(x).T @ fp22(2k). The PE
         truncates fp32 operands to fp22 (12-bit mantissa); both operands
         are pre-rounded to fp22 RNE on the host so the truncation is
         exact. One extra mantissa bit vs fp16 on both sides.
      3. 4x fp8 DoubleRow matmuls (K=256 packed, N=512) for the key
         residual cross term e4m3(x) . e5m2(kl22), kl22 = 2k - fp22(2k).
         e5m2 holds the tiny residuals without subnormal loss (normals
         from 2^-14), e4m3 holds x raw; product scale is exactly 1 so the
         pass accumulates directly onto the score PSUM.
      4. (S14 only) 2x fp8 DoubleRow matmuls for the query residual
         cross e5m2(xl22) . e4m3(2k) over the first 256 dims.
    A matmul instruction costs ~N cycles regardless of dtype, so the tile
    costs 12 (S12) or 14 (S14) matmuls vs 16 for the fp16+2-cross scheme.
    Numpy-sim of the exact arithmetic: S12 flips 9 of 65536 argmaxes
    (rel 1.62e-2), S14 flips 4 (rel 1.09e-2); gate 2e-2.
  - DVE per tile: MAX8 + FIND_INDEX8 read straight from PSUM, then a
    gpsimd indirect-DMA gathers fp16 values rows and the result is
    written out from the Scalar queue; the host upcasts to f32.
"""

import numpy as np

_B = 65536
_D = 512
_K = 1024
_NCORES = 8
_BL = _B // _NCORES  # 8192 rows per core
_P = 128
_BBLK = 512          # b columns loaded per DMA
_BT = 128            # b rows per matmul tile (PSUM partition dim)
_DC = _D // _P       # 4 contraction chunks

_USE_QHALF = True    # S14 (True): + query-residual half cross; S12 (False)

_cached = None


def _build():
    import concourse.mybir as mybir
    from concourse import bacc
    from concourse.bass import IndirectOffsetOnAxis
    from concourse.tile import TileContext

    f32 = mybir.dt.float32
    f32r = mybir.dt.float32r
    f16 = mybir.dt.float16
    e4 = mybir.dt.float8e4
    e5 = mybir.dt.float8e5
    u32 = mybir.dt.uint32
    bf16 = mybir.dt.bfloat16
    DR = mybir.MatmulPerfMode.DoubleRow

    nc = bacc.Bacc("TRN2", target_bir_lowering=False, debug=False,
                   num_devices=_NCORES)
    x22 = nc.dram_tensor("x22", [_D, _BL], f32r, kind="ExternalInput")
    xe4 = nc.dram_tensor("xe4", [_D, _BL], e4, kind="ExternalInput")
    k22 = nc.dram_tensor("k22", [_D, _K], f32r, kind="ExternalInput")
    kl5 = nc.dram_tensor("kl5", [_D, _K], e5, kind="ExternalInput")
    biasf = nc.dram_tensor("biasf", [_P, _K], f32, kind="ExternalInput")
    vals = nc.dram_tensor("vals", [_K, _D], f16, kind="ExternalInput")
    if _USE_QHALF:
        xl5 = nc.dram_tensor("xl5", [_D // 2, _BL], e5, kind="ExternalInput")
        k4h = nc.dram_tensor("k4h", [_D // 2, _K], e4, kind="ExternalInput")
    out = nc.dram_tensor("out", [_BL, _D], f16, kind="ExternalOutput")

    x22_3 = x22.rearrange("(do p) b -> p do b", p=_P)    # [128, 4, 8192]
    xe4_4 = xe4.rearrange("(c j p) b -> p c j b", p=_P, c=2, j=2)
    k22_3 = k22.rearrange("(do p) k -> p do k", p=_P)    # [128, 4, 1024]
    kl5_4 = kl5.rearrange("(c j p) k -> p c j k", p=_P, c=2, j=2)
    if _USE_QHALF:
        xl5_3 = xl5.rearrange("(j p) b -> p j b", p=_P, j=2)
        k4h_3 = k4h.rearrange("(j p) k -> p j k", p=_P, j=2)

    with TileContext(nc) as tc:
        with (
            tc.tile_pool(name="const", bufs=1) as cpool,
            tc.tile_pool(name="xp", bufs=3) as xpool,
            tc.tile_pool(name="warm", bufs=1) as warmpool,
            tc.tile_pool(name="st", bufs=4) as stpool,
            tc.tile_pool(name="gp", bufs=4) as gpool,
            tc.tile_pool(name="ps", bufs=3, space="PSUM") as pspool,
            tc.tile_pool(name="wps", bufs=1, space="PSUM") as wpspool,
        ):
            # Const loads on the Scalar engine's HWDGE queue, ordered by
            # first consumption (tile0 half0 first).
            bias_sb = cpool.tile([_P, _K], f32)
            k22_sb = cpool.tile([_P, _DC, _K], f32r)
            kl5_sb = cpool.tile([_P, 2, 2, _K], e5)
            if _USE_QHALF:
                k4h_sb = cpool.tile([_P, 2, _K], e4)
            nc.scalar.dma_start(bias_sb[:], biasf[:, :])
            nc.scalar.dma_start(k22_sb[:, :, 0:512], k22_3[:, :, 0:512])
            nc.scalar.dma_start(kl5_sb[:, :, :, 0:512], kl5_4[:, :, :, 0:512])
            if _USE_QHALF:
                nc.scalar.dma_start(k4h_sb[:, :, 0:512], k4h_3[:, :, 0:512])
            nc.scalar.dma_start(k22_sb[:, :, 512:1024], k22_3[:, :, 512:1024])
            nc.scalar.dma_start(kl5_sb[:, :, :, 512:1024], kl5_4[:, :, :, 512:1024])
            if _USE_QHALF:
                nc.scalar.dma_start(k4h_sb[:, :, 512:1024], k4h_3[:, :, 512:1024])

            # Pre-warm the PE clock (HAM) during the initial DMA wait.
            wsrc = warmpool.tile([_P, 64], bf16)
            nc.vector.memset(wsrc[:], 0.0)
            wps = wpspool.tile([_P, 64], f32)
            for _ in range(66):
                nc.tensor.matmul(wps[:64, :], lhsT=wsrc[:, :64], rhs=wsrc[:],
                                 start=True, stop=True)
            # Pre-touch all 3 rotating score PSUM buffers with start=True
            # matmuls so their has_written bits are set: steady-state tiles
            # never use start=True (the bias is pre-written by ScalarE and
            # matmuls accumulate onto it), and a PE write with has_written=0
            # would overwrite the bias instead of accumulating.
            wlhs = warmpool.tile([_P, _P], bf16)
            nc.vector.memset(wlhs[:], 0.0)
            wrhs = warmpool.tile([_P, 512], bf16)
            nc.vector.memset(wrhs[:], 0.0)
            for _ in range(3):
                wtile = pspool.tile([_P, _K], f32, tag="ps")
                nc.tensor.matmul(wtile[:, 0:512], lhsT=wlhs[:],
                                 rhs=wrhs[:], start=True, stop=True)
                nc.tensor.matmul(wtile[:, 512:1024], lhsT=wlhs[:],
                                 rhs=wrhs[:], start=True, stop=True)

            blocks = [(0, _BT)]
            off = _BT
            while off < _BL:
                w = min(_BBLK, _BL - off)
                blocks.append((off, w))
                off += w

            for boff, bw in blocks:
                x22_t = xpool.tile([_P, _DC, _BBLK], f32r, tag="xh")
                xe4_t = xpool.tile([_P, 2, 2, _BBLK], e4, tag="xe")
                nc.sync.dma_start(x22_t[:, :, :bw], x22_3[:, :, boff:boff + bw])
                nc.sync.dma_start(xe4_t[:, :, :, :bw], xe4_4[:, :, :, boff:boff + bw])
                if _USE_QHALF:
                    xl5_t = xpool.tile([_P, 2, _BBLK], e5, tag="xl")
                    nc.sync.dma_start(xl5_t[:, :, :bw], xl5_3[:, :, boff:boff + bw])

                for sub in range(bw // _BT):
                    bt = boff // _BT + sub
                    bsl = slice(sub * _BT, (sub + 1) * _BT)
                    ps = pspool.tile([_P, _K], f32, tag="ps")
                    hs = [slice(0, 512), slice(512, 1024)]
                    # ScalarE pre-writes the exact f32 -|k|^2 bias into PSUM;
                    # all matmuls then accumulate onto it (start=False; the
                    # has_written bits were set once by the warmup matmuls).
                    # Matmuls grouped by PE mode (fp32r then fp8-DR) to
                    # minimize mode-switch stalls; within the fp32r group,
                    # dc outer / h inner so consecutive matmuls share the
                    # same stationary operand.
                    nc.scalar.copy(out=ps[:], in_=bias_sb[:])
                    for dc in range(_DC):
                        for h in range(2):
                            nc.tensor.matmul(ps[:, hs[h]], lhsT=x22_t[:, dc, bsl],
                                             rhs=k22_sb[:, dc, hs[h]],
                                             start=False, stop=False,
                                             skip_group_check=True)
                    last = (not _USE_QHALF)
                    for c in range(2):
                        for h in range(2):
                            nc.tensor.matmul(ps[:, hs[h]], lhsT=xe4_t[:, c, :, bsl],
                                             rhs=kl5_sb[:, c, :, hs[h]],
                                             perf_mode=DR,
                                             start=False, stop=(last and c == 1),
                                             skip_group_check=True)
                    if _USE_QHALF:
                        for h in range(2):
                            nc.tensor.matmul(ps[:, hs[h]], lhsT=xl5_t[:, :, bsl],
                                             rhs=k4h_sb[:, :, hs[h]],
                                             perf_mode=DR,
                                             start=False, stop=True,
                                             skip_group_check=True)
                    mx = stpool.tile([_P, 8], f32)
                    nc.vector.max(out=mx[:], in_=ps[:])
                    idx = stpool.tile([_P, 8], u32)
                    nc.vector.max_index(out=idx[:], in_max=mx[:], in_values=ps[:])

                    g = gpool.tile([_P, _D], f16)
                    nc.gpsimd.indirect_dma_start(
                        out=g[:],
                        out_offset=None,
                        in_=vals[:, :],
                        in_offset=IndirectOffsetOnAxis(ap=idx[:, :1], axis=0),
                    )
                    nc.scalar.dma_start(out[bt * _BT:(bt + 1) * _BT, :], g[:])

    nc.compile()
    return nc


def _get_nc():
    global _cached
    if _cached is None:
        _cached = _build()
    return _cached


def _fp22(a):
    """Round f32 to 12-bit mantissa (fp22) RNE, so the PE's fp32r
    truncation is exact."""
    u = np.ascontiguousarray(a, dtype=np.float32).view(np.uint32)
    r = (u + 0x7FF + ((u >> 12) & 1)) & np.uint32(0xFFFFF000)
    return r.view(np.float32)


def _e4(a):
    import ml_dtypes

    return np.clip(a, -240.0, 240.0).astype(ml_dtypes.float8_e4m3)


def _e5(a):
    import ml_dtypes

    return np.clip(a, -57344.0, 57344.0).astype(ml_dtypes.float8_e5m2)


def _prepare_in_maps(x, keys, values):
    x = np.asarray(x, dtype=np.float32)
    keys = np.asarray(keys, dtype=np.float32)
    values = np.asarray(values, dtype=np.float32)

    kT = np.ascontiguousarray((2.0 * keys).T)            # [512, 1024] f32
    k22 = _fp22(kT)
    kl22 = kT - k22                                      # exact in f32
    kl5 = _e5(kl22)
    k4h = _e4(kT[: _D // 2])

    k2 = np.einsum("kd,kd->k", keys.astype(np.float64),
                   keys.astype(np.float64))
    biasf = np.ascontiguousarray(
        np.broadcast_to((-k2).astype(np.float32), (_P, _K)))
    vals16 = values.astype(np.float16)

    in_maps = []
    for c in range(_NCORES):
        xs = np.ascontiguousarray(x[c * _BL:(c + 1) * _BL].T)  # [512, 8192]
        x22 = _fp22(xs)
        m = {
            "x22": x22, "xe4": _e4(xs),
            "k22": k22, "kl5": kl5,
            "biasf": biasf, "vals": vals16,
        }
        if _USE_QHALF:
            m["xl5"] = _e5(xs[: _D // 2] - x22[: _D // 2])
            m["k4h"] = k4h
        in_maps.append(m)
    return in_maps


def kernel(x, keys, values):
    from concourse.bass_utils import run_bass_kernel_spmd

    nc = _get_nc()
    in_maps = _prepare_in_maps(x, keys, values)
    res = run_bass_kernel_spmd(nc, in_maps, core_ids=list(range(_NCORES)))
    return np.concatenate([r["out"] for r in res.results],
                          axis=0).astype(np.float32)


# revision 3
# speedup vs baseline: 1.1746x; 1.1746x over previous
"""VQ codebook lookup kernel for Trainium2 (8 NeuronCores, data-parallel).

Computes out[b] = values[argmin_k ||x[b] - keys[k]||] for
x [65536, 512], keys/values [1024, 512] fp32.

Strategy (per core, batch shard of 8192 rows):
  - argmin distance == argmax s = 2*x.k - |k|^2 (sqrt and the |x|^2 row
    offset do not change the argmin).
  - Per 128-row tile, s[128, 1024] accumulates in PSUM from:
      1. ScalarE pre-writes the exact f32 -|k|^2 bias into the PSUM tile;
         all matmuls use start=False and accumulate onto it (warmup
         matmuls pre-set the has_written bits of the 4 rotating PSUM
         buffers once, so the PE accumulates instead of overwriting).
      2. 8x fp16 matmuls (K=128, N=512): fp16(x).T @ fp16(2k) hi*hi term.
         The PE multiplies fp16 operands exactly (fp22 internally) and
         accumulates in fp32.
      3. 8x fp8-e4m3 DoubleRow matmuls (K=256 packed, N=512) for the two
         cross terms: fp8(xl*64) . fp8(2k/64) and fp8(x/32) . fp8(kl*32),
         where xl = x - fp16(x), kl = 2k - fp16(2k); scales keep the
         operands inside e4m3 range. DoubleRow packs 2 contraction rows
         per PE cell, halving the matmul count for these passes.
    A matmul instruction costs ~N cycles regardless of dtype, so the tile
    costs 16 matmuls vs 24 for the bf16 hi/lo x3 scheme (1.5x less PE
    time); matmuls are grouped by PE mode (fp16, then DoubleRow) to
    minimize mode-switch stalls. Numpy-sim of this exact arithmetic
    flips 1 of 65536 argmaxes (rel err 5.5e-3, gate 2e-2).
  - DVE per tile: MAX8 + FIND_INDEX8 read straight from PSUM (no
    PSUM->SBUF move), then a gpsimd indirect-DMA gathers fp16 values
    rows and the result is written out from the Scalar queue; the host
    upcasts the fp16 output to f32 (values fp16 rounding adds ~1e-3 rel,
    far under the gate, and halves the gather+store HBM traffic).
"""

import numpy as np

_B = 65536
_D = 512
_K = 1024
_NCORES = 8
_BL = _B // _NCORES  # 8192 rows per core
_P = 128
_BBLK = 512          # b columns loaded per DMA
_BT = 128            # b rows per matmul tile (PSUM partition dim)
_DC = _D // _P       # 4 contraction chunks
_AX = 64.0           # scale for xl-cross fp8 pass
_AK = 32.0           # scale for kl-cross fp8 pass
_NPS = 4             # rotating score PSUM buffers (4 x 2 banks = all 8)

_cached = None


def _build():
    import concourse.mybir as mybir
    from concourse import bacc
    from concourse.bass import IndirectOffsetOnAxis
    from concourse.tile import TileContext

    f32 = mybir.dt.float32
    f16 = mybir.dt.float16
    f8 = mybir.dt.float8e4
    u32 = mybir.dt.uint32
    bf16 = mybir.dt.bfloat16
    DR = mybir.MatmulPerfMode.DoubleRow

    nc = bacc.Bacc("TRN2", target_bir_lowering=False, debug=False,
                   num_devices=_NCORES)
    xh16 = nc.dram_tensor("xh16", [_D, _BL], f16, kind="ExternalInput")
    xl8 = nc.dram_tensor("xl8", [_D, _BL], f8, kind="ExternalInput")
    xf8 = nc.dram_tensor("xf8", [_D, _BL], f8, kind="ExternalInput")
    k16 = nc.dram_tensor("k16", [_D, _K], f16, kind="ExternalInput")
    k8 = nc.dram_tensor("k8", [_D, _K], f8, kind="ExternalInput")
    kl8 = nc.dram_tensor("kl8", [_D, _K], f8, kind="ExternalInput")
    biasf = nc.dram_tensor("biasf", [_P, _K], f32, kind="ExternalInput")
    vals = nc.dram_tensor("vals", [_K, _D], f16, kind="ExternalInput")
    out = nc.dram_tensor("out", [_BL, _D], f16, kind="ExternalOutput")

    xh3 = xh16.rearrange("(do p) b -> p do b", p=_P)    # [128, 4, 8192]
    xl4 = xl8.rearrange("(c j p) b -> p c j b", p=_P, c=2, j=2)
    xf4 = xf8.rearrange("(c j p) b -> p c j b", p=_P, c=2, j=2)
    k16_3 = k16.rearrange("(do p) k -> p do k", p=_P)   # [128, 4, 1024]
    k8_4 = k8.rearrange("(c j p) k -> p c j k", p=_P, c=2, j=2)
    kl8_4 = kl8.rearrange("(c j p) k -> p c j k", p=_P, c=2, j=2)

    with TileContext(nc) as tc:
        with (
            tc.tile_pool(name="const", bufs=1) as cpool,
            tc.tile_pool(name="xp", bufs=3) as xpool,
            tc.tile_pool(name="warm", bufs=1) as warmpool,
            tc.tile_pool(name="st", bufs=4) as stpool,
            tc.tile_pool(name="gp", bufs=4) as gpool,
            tc.tile_pool(name="ps", bufs=_NPS, space="PSUM") as pspool,
        ):
            # Const loads on the Scalar engine's HWDGE queue, ordered by
            # first consumption (tile0 half0 first).
            bias_sb = cpool.tile([_P, _K], f32)
            k16_sb = cpool.tile([_P, _DC, _K], f16)
            k8_sb = cpool.tile([_P, 2, 2, _K], f8)
            kl8_sb = cpool.tile([_P, 2, 2, _K], f8)
            nc.scalar.dma_start(bias_sb[:], biasf[:, :])
            nc.scalar.dma_start(k16_sb[:, :, 0:512], k16_3[:, :, 0:512])
            nc.scalar.dma_start(k8_sb[:, :, :, 0:512], k8_4[:, :, :, 0:512])
            nc.scalar.dma_start(kl8_sb[:, :, :, 0:512], kl8_4[:, :, :, 0:512])
            nc.scalar.dma_start(k16_sb[:, :, 512:1024], k16_3[:, :, 512:1024])
            nc.scalar.dma_start(k8_sb[:, :, :, 512:1024], k8_4[:, :, :, 512:1024])
            nc.scalar.dma_start(kl8_sb[:, :, :, 512:1024], kl8_4[:, :, :, 512:1024])

            # Warmup operands memset on GpSimd (its engine comes up ~1.5us
            # before VectorE, so the PE clock warmup starts sooner).
            wsrc = warmpool.tile([_P, 64], bf16)
            nc.gpsimd.memset(wsrc[:], 0.0)
            wlhs = warmpool.tile([_P, _P], bf16)
            nc.gpsimd.memset(wlhs[:], 0.0)
            wrhs = warmpool.tile([_P, 512], bf16)
            nc.gpsimd.memset(wrhs[:], 0.0)

            # Pre-touch all rotating score PSUM buffers with start=True
            # matmuls so their has_written bits are set: steady-state tiles
            # never use start=True (the bias is pre-written by ScalarE and
            # matmuls accumulate onto it), and a PE write with has_written=0
            # would overwrite the bias instead of accumulating. The first
            # buffer also hosts the PE clock (HAM) warmup matmuls, which run
            # during the initial const-DMA wait.
            for b in range(_NPS):
                wtile = pspool.tile([_P, _K], f32, tag="ps")
                if b == 0:
                    for _ in range(66):
                        nc.tensor.matmul(wtile[:64, 0:64], lhsT=wsrc[:, :64],
                                         rhs=wsrc[:], start=True, stop=True)
                nc.tensor.matmul(wtile[:, 0:512], lhsT=wlhs[:],
                                 rhs=wrhs[:], start=True, stop=True)
                nc.tensor.matmul(wtile[:, 512:1024], lhsT=wlhs[:],
                                 rhs=wrhs[:], start=True, stop=True)

            blocks = [(0, _BT)]
            off = _BT
            while off < _BL:
                w = min(_BBLK, _BL - off)
                blocks.append((off, w))
                off += w

            for boff, bw in blocks:
                xh_t = xpool.tile([_P, _DC, _BBLK], f16, tag="xh")
                xl_t = xpool.tile([_P, 2, 2, _BBLK], f8, tag="xl")
                xf_t = xpool.tile([_P, 2, 2, _BBLK], f8, tag="xf")
                nc.sync.dma_start(xh_t[:, :, :bw], xh3[:, :, boff:boff + bw])
                nc.sync.dma_start(xl_t[:, :, :, :bw], xl4[:, :, :, boff:boff + bw])
                nc.sync.dma_start(xf_t[:, :, :, :bw], xf4[:, :, :, boff:boff + bw])

                for sub in range(bw // _BT):
                    bt = boff // _BT + sub
                    bsl = slice(sub * _BT, (sub + 1) * _BT)
                    ps = pspool.tile([_P, _K], f32, tag="ps")
                    hs = [slice(0, 512), slice(512, 1024)]
                    # ScalarE pre-writes the exact f32 -|k|^2 bias into PSUM;
                    # all matmuls then accumulate onto it (start=False; the
                    # has_written bits were set once by the warmup matmuls).
                    # Matmuls grouped by PE mode (fp16 then fp8-DR) to
                    # minimize mode-switch stalls; within the fp16 group,
                    # dc outer / h inner so consecutive matmuls share the
                    # same stationary operand.
                    nc.scalar.copy(out=ps[:], in_=bias_sb[:])
                    for dc in range(_DC):
                        for h in range(2):
                            nc.tensor.matmul(ps[:, hs[h]], lhsT=xh_t[:, dc, bsl],
                                             rhs=k16_sb[:, dc, hs[h]],
                                             start=False, stop=False,
                                             skip_group_check=True)
                    for c in range(2):
                        for h in range(2):
                            nc.tensor.matmul(ps[:, hs[h]], lhsT=xl_t[:, c, :, bsl],
                                             rhs=k8_sb[:, c, :, hs[h]],
                                             perf_mode=DR,
                                             start=False, stop=False,
                                             skip_group_check=True)
                    for c in range(2):
                        for h in range(2):
                            nc.tensor.matmul(ps[:, hs[h]], lhsT=xf_t[:, c, :, bsl],
                                             rhs=kl8_sb[:, c, :, hs[h]],
                                             perf_mode=DR,
                                             start=False, stop=(c == 1),
                                             skip_group_check=True)
                    mx = stpool.tile([_P, 8], f32)
                    nc.vector.max(out=mx[:], in_=ps[:])
                    idx = stpool.tile([_P, 8], u32)
                    nc.vector.max_index(out=idx[:], in_max=mx[:], in_values=ps[:])

                    g = gpool.tile([_P, _D], f16)
                    nc.gpsimd.indirect_dma_start(
                        out=g[:],
                        out_offset=None,
                        in_=vals[:, :],
                        in_offset=IndirectOffsetOnAxis(ap=idx[:, :1], axis=0),
                    )
                    nc.scalar.dma_start(out[bt * _BT:(bt + 1) * _BT, :], g[:])

    nc.compile()
    return nc


def _get_nc():
    global _cached
    if _cached is None:
        _cached = _build()
    return _cached


def _fp8(a):
    import ml_dtypes

    return np.clip(a, -240.0, 240.0).astype(ml_dtypes.float8_e4m3)


def _prepare_in_maps(x, keys, values):
    x = np.asarray(x, dtype=np.float32)
    keys = np.asarray(keys, dtype=np.float32)
    values = np.asarray(values, dtype=np.float32)

    kT = np.ascontiguousarray((2.0 * keys).T)            # [512, 1024] f32
    k16 = kT.astype(np.float16)
    kl = kT - k16.astype(np.float32)
    k8 = _fp8(kT / _AX)
    kl8 = _fp8(kl * _AK)

    k2 = np.einsum("kd,kd->k", keys.astype(np.float64),
                   keys.astype(np.float64))
    biasf = np.ascontiguousarray(
        np.broadcast_to((-k2).astype(np.float32), (_P, _K)))
    vals16 = values.astype(np.float16)

    in_maps = []
    for c in range(_NCORES):
        xs = np.ascontiguousarray(x[c * _BL:(c + 1) * _BL].T)  # [512, 8192]
        xh16 = xs.astype(np.float16)
        xl = xs - xh16.astype(np.float32)
        xl8 = _fp8(xl * _AX)
        xf8 = _fp8(xs / _AK)
        in_maps.append({
            "xh16": xh16, "xl8": xl8, "xf8": xf8,
            "k16": k16, "k8": k8, "kl8": kl8,
            "biasf": biasf, "vals": vals16,
        })
    return in_maps


def kernel(x, keys, values):
    from concourse.bass_utils import run_bass_kernel_spmd

    nc = _get_nc()
    in_maps = _prepare_in_maps(x, keys, values)
    res = run_bass_kernel_spmd(nc, in_maps, core_ids=list(range(_NCORES)))
    return np.concatenate([r["out"] for r in res.results],
                          axis=0).astype(np.float32)


# revision 8
# speedup vs baseline: 1.1917x; 1.0146x over previous
"""VQ codebook lookup kernel for Trainium2 (8 NeuronCores, data-parallel).

Computes out[b] = values[argmin_k ||x[b] - keys[k]||] for
x [65536, 512], keys/values [1024, 512] fp32.

Strategy (per core, batch shard of 8192 rows):
  - argmin distance == argmax s = 2*x.k - |k|^2 (sqrt and the |x|^2 row
    offset do not change the argmin).
  - Per 128-row tile, s[128, 1024] accumulates in PSUM from:
      1. ScalarE pre-writes the exact f32 -|k|^2 bias into the PSUM tile;
         all matmuls use start=False and accumulate onto it (warmup
         matmuls pre-set the has_written bits of the 4 rotating PSUM
         buffers once, so the PE accumulates instead of overwriting).
      2. 8x fp16 matmuls (K=128, N=512): fp16(x).T @ fp16(2k) hi*hi term.
         The PE multiplies fp16 operands exactly (fp22 internally) and
         accumulates in fp32.
      3. 8x fp8-e4m3 DoubleRow matmuls (K=256 packed, N=512) for the two
         cross terms: fp8(xl*64) . fp8(2k/64) and fp8(x/32) . fp8(kl*32),
         where xl = x - fp16(x), kl = 2k - fp16(2k); scales keep the
         operands inside e4m3 range. DoubleRow packs 2 contraction rows
         per PE cell, halving the matmul count for these passes.
    A matmul instruction costs ~N cycles regardless of dtype, so the tile
    costs 16 matmuls vs 24 for the bf16 hi/lo x3 scheme (1.5x less PE
    time); matmuls are grouped by PE mode (fp16, then DoubleRow) to
    minimize mode-switch stalls. Numpy-sim of this exact arithmetic
    flips 1 of 65536 argmaxes (rel err 5.5e-3, gate 2e-2).
  - DVE per tile: MAX8 + FIND_INDEX8 read straight from PSUM (no
    PSUM->SBUF move), then a gpsimd indirect-DMA gathers fp16 values
    rows and the result is written out from the Scalar queue; the host
    upcasts the fp16 output to f32 (values fp16 rounding adds ~1e-3 rel,
    far under the gate, and halves the gather+store HBM traffic).
"""

import numpy as np

_B = 65536
_D = 512
_K = 1024
_NCORES = 8
_BL = _B // _NCORES  # 8192 rows per core
_P = 128
_BBLK = 512          # b columns loaded per DMA
_BT = 128            # b rows per matmul tile (PSUM partition dim)
_DC = _D // _P       # 4 contraction chunks
_AX = 64.0           # scale for xl-cross fp8 pass
_AK = 32.0           # scale for kl-cross fp8 pass
_NPS = 4             # rotating score PSUM buffers (4 x 2 banks = all 8)

_cached = None


def _build():
    import concourse.mybir as mybir
    from concourse import bacc
    from concourse.bass import IndirectOffsetOnAxis
    from concourse.tile import TileContext

    f32 = mybir.dt.float32
    f16 = mybir.dt.float16
    f8 = mybir.dt.float8e4
    u32 = mybir.dt.uint32
    bf16 = mybir.dt.bfloat16
    DR = mybir.MatmulPerfMode.DoubleRow

    nc = bacc.Bacc("TRN2", target_bir_lowering=False, debug=False,
                   num_devices=_NCORES)
    xh16 = nc.dram_tensor("xh16", [_D, _BL], f16, kind="ExternalInput")
    xl8 = nc.dram_tensor("xl8", [_D, _BL], f8, kind="ExternalInput")
    xf8 = nc.dram_tensor("xf8", [_D, _BL], f8, kind="ExternalInput")
    k16 = nc.dram_tensor("k16", [_D, _K], f16, kind="ExternalInput")
    k8 = nc.dram_tensor("k8", [_D, _K], f8, kind="ExternalInput")
    kl8 = nc.dram_tensor("kl8", [_D, _K], f8, kind="ExternalInput")
    biasf = nc.dram_tensor("biasf", [1, _K], f32, kind="ExternalInput")
    vals = nc.dram_tensor("vals", [_K, _D], f16, kind="ExternalInput")
    out = nc.dram_tensor("out", [_BL, _D], f16, kind="ExternalOutput")

    xh3 = xh16.rearrange("(do p) b -> p do b", p=_P)    # [128, 4, 8192]
    xl4 = xl8.rearrange("(c j p) b -> p c j b", p=_P, c=2, j=2)
    xf4 = xf8.rearrange("(c j p) b -> p c j b", p=_P, c=2, j=2)
    k16_3 = k16.rearrange("(do p) k -> p do k", p=_P)   # [128, 4, 1024]
    k8_4 = k8.rearrange("(c j p) k -> p c j k", p=_P, c=2, j=2)
    kl8_4 = kl8.rearrange("(c j p) k -> p c j k", p=_P, c=2, j=2)

    with TileContext(nc) as tc:
        with (
            tc.tile_pool(name="const", bufs=1) as cpool,
            tc.tile_pool(name="xp", bufs=3) as xpool,
            tc.tile_pool(name="warm", bufs=1) as warmpool,
            tc.tile_pool(name="st", bufs=4) as stpool,
            tc.tile_pool(name="gp", bufs=4) as gpool,
            tc.tile_pool(name="ps", bufs=_NPS, space="PSUM") as pspool,
        ):
            # All input loads share the Sync engine's HWDGE queue so they
            # drain in consumption order (tile0's needs first); split queues
            # let the x-block prefetch flood starve the tiny const loads.
            bias_row = cpool.tile([1, _K], f32)
            bias_sb = cpool.tile([_P, _K], f32)
            k16_sb = cpool.tile([_P, _DC, _K], f16)
            k8_sb = cpool.tile([_P, 2, 2, _K], f8)
            kl8_sb = cpool.tile([_P, 2, 2, _K], f8)
            nc.sync.dma_start(bias_row[:], biasf[:, :])
            nc.sync.dma_start(k16_sb[:, :, 0:512], k16_3[:, :, 0:512])
            nc.sync.dma_start(k16_sb[:, :, 512:1024], k16_3[:, :, 512:1024])
            nc.sync.dma_start(k8_sb[:, :, :, 0:512], k8_4[:, :, :, 0:512])
            nc.sync.dma_start(kl8_sb[:, :, :, 0:512], kl8_4[:, :, :, 0:512])

            # Warmup operands memset on GpSimd (its engine comes up ~1.5us
            # before VectorE).
            ones = warmpool.tile([1, _P], f32)
            nc.gpsimd.memset(ones[:], 1.0)
            wlhs = warmpool.tile([_P, _P], bf16)
            nc.gpsimd.memset(wlhs[:], 0.0)
            wrhs = warmpool.tile([_P, 512], bf16)
            nc.gpsimd.memset(wrhs[:], 0.0)

            # Broadcast the 4KB bias row to all 128 partitions with a pair
            # of K=1 fp32 matmuls (ones.T @ bias_row, exact), then copy
            # PSUM -> SBUF. The two cold fp32 4-pass matmuls double as the
            # PE clock (HAM) warmup. Afterwards pre-touch all rotating
            # score PSUM buffers with start=True matmuls so their
            # has_written bits are set: steady-state tiles never use
            # start=True (the bias is pre-written by ScalarE and matmuls
            # accumulate onto it), and a PE write with has_written=0 would
            # overwrite the bias instead of accumulating.
            btile = pspool.tile([_P, _K], f32, tag="ps")
            nc.tensor.matmul(btile[:, 0:512], lhsT=ones[:],
                             rhs=bias_row[:, 0:512], start=True, stop=True)
            nc.tensor.matmul(btile[:, 512:1024], lhsT=ones[:],
                             rhs=bias_row[:, 512:1024], start=True, stop=True)
            nc.scalar.copy(out=bias_sb[:], in_=btile[:])
            for b in range(_NPS):
                wtile = pspool.tile([_P, _K], f32, tag="ps")
                nc.tensor.matmul(wtile[:, 0:512], lhsT=wlhs[:],
                                 rhs=wrhs[:], start=True, stop=True)
                nc.tensor.matmul(wtile[:, 512:1024], lhsT=wlhs[:],
                                 rhs=wrhs[:], start=True, stop=True)

            # Remaining const halves, queued behind tile0's critical loads
            # but ahead of the bulk x prefetch below.
            nc.sync.dma_start(k8_sb[:, :, :, 512:1024], k8_4[:, :, :, 512:1024])
            nc.sync.dma_start(kl8_sb[:, :, :, 512:1024], kl8_4[:, :, :, 512:1024])

            blocks = [(0, _BT)]
            off = _BT
            while off < _BL:
                w = min(_BBLK, _BL - off)
                blocks.append((off, w))
                off += w

            for boff, bw in blocks:
                xh_t = xpool.tile([_P, _DC, _BBLK], f16, tag="xh")
                xl_t = xpool.tile([_P, 2, 2, _BBLK], f8, tag="xl")
                xf_t = xpool.tile([_P, 2, 2, _BBLK], f8, tag="xf")
                nc.sync.dma_start(xh_t[:, :, :bw], xh3[:, :, boff:boff + bw])
                nc.sync.dma_start(xl_t[:, :, :, :bw], xl4[:, :, :, boff:boff + bw])
                nc.sync.dma_start(xf_t[:, :, :, :bw], xf4[:, :, :, boff:boff + bw])

                for sub in range(bw // _BT):
                    bt = boff // _BT + sub
                    bsl = slice(sub * _BT, (sub + 1) * _BT)
                    ps = pspool.tile([_P, _K], f32, tag="ps")
                    hs = [slice(0, 512), slice(512, 1024)]
                    # ScalarE pre-writes the exact f32 -|k|^2 bias into PSUM;
                    # all matmuls then accumulate onto it (start=False; the
                    # has_written bits were set once by the warmup matmuls).
                    # Matmuls grouped by PE mode (fp16 then fp8-DR) to
                    # minimize mode-switch stalls; within the fp16 group,
                    # dc outer / h inner so consecutive matmuls share the
                    # same stationary operand.
                    nc.scalar.copy(out=ps[:], in_=bias_sb[:])
                    for dc in range(_DC):
                        for h in range(2):
                            nc.tensor.matmul(ps[:, hs[h]], lhsT=xh_t[:, dc, bsl],
                                             rhs=k16_sb[:, dc, hs[h]],
                                             start=False, stop=False,
                                             skip_group_check=True)
                    for c in range(2):
                        for h in range(2):
                            nc.tensor.matmul(ps[:, hs[h]], lhsT=xl_t[:, c, :, bsl],
                                             rhs=k8_sb[:, c, :, hs[h]],
                                             perf_mode=DR,
                                             start=False, stop=False,
                                             skip_group_check=True)
                    for c in range(2):
                        for h in range(2):
                            nc.tensor.matmul(ps[:, hs[h]], lhsT=xf_t[:, c, :, bsl],
                                             rhs=kl8_sb[:, c, :, hs[h]],
                                             perf_mode=DR,
                                             start=False, stop=(c == 1),
                                             skip_group_check=True)
                    mx = stpool.tile([_P, 8], f32)
                    nc.vector.max(out=mx[:], in_=ps[:])
                    idx = stpool.tile([_P, 8], u32)
                    nc.vector.max_index(out=idx[:], in_max=mx[:], in_values=ps[:])

                    g = gpool.tile([_P, _D], f16)
                    nc.gpsimd.indirect_dma_start(
                        out=g[:],
                        out_offset=None,
                        in_=vals[:, :],
                        in_offset=IndirectOffsetOnAxis(ap=idx[:, :1], axis=0),
                    )
                    nc.scalar.dma_start(out[bt * _BT:(bt + 1) * _BT, :], g[:])

    nc.compile()
    return nc


def _get_nc():
    global _cached
    if _cached is None:
        _cached = _build()
    return _cached


def _fp8(a):
    import ml_dtypes

    return np.clip(a, -240.0, 240.0).astype(ml_dtypes.float8_e4m3)


def _prepare_in_maps(x, keys, values):
    x = np.asarray(x, dtype=np.float32)
    keys = np.asarray(keys, dtype=np.float32)
    values = np.asarray(values, dtype=np.float32)

    kT = np.ascontiguousarray((2.0 * keys).T)            # [512, 1024] f32
    k16 = kT.astype(np.float16)
    kl = kT - k16.astype(np.float32)
    k8 = _fp8(kT / _AX)
    kl8 = _fp8(kl * _AK)

    k2 = np.einsum("kd,kd->k", keys.astype(np.float64),
                   keys.astype(np.float64))
    biasf = np.ascontiguousarray((-k2).astype(np.float32)[None, :])
    vals16 = values.astype(np.float16)

    in_maps = []
    for c in range(_NCORES):
        xs = np.ascontiguousarray(x[c * _BL:(c + 1) * _BL].T)  # [512, 8192]
        xh16 = xs.astype(np.float16)
        xl = xs - xh16.astype(np.float32)
        xl8 = _fp8(xl * _AX)
        xf8 = _fp8(xs / _AK)
        in_maps.append({
            "xh16": xh16, "xl8": xl8, "xf8": xf8,
            "k16": k16, "k8": k8, "kl8": kl8,
            "biasf": biasf, "vals": vals16,
        })
    return in_maps


def kernel(x, keys, values):
    from concourse.bass_utils import run_bass_kernel_spmd

    nc = _get_nc()
    in_maps = _prepare_in_maps(x, keys, values)
    res = run_bass_kernel_spmd(nc, in_maps, core_ids=list(range(_NCORES)))
    return np.concatenate([r["out"] for r in res.results],
                          axis=0).astype(np.float32)


# revision 13
# speedup vs baseline: 1.1989x; 1.0060x over previous
"""VQ codebook lookup kernel for Trainium2 (8 NeuronCores, data-parallel).

Computes out[b] = values[argmin_k ||x[b] - keys[k]||] for
x [65536, 512], keys/values [1024, 512] fp32.

Strategy (per core, batch shard of 8192 rows):
  - argmin distance == argmax s = 2*x.k - |k|^2 (sqrt and the |x|^2 row
    offset do not change the argmin).
  - Per 128-row tile, s[128, 1024] accumulates in PSUM from:
      1. ScalarE pre-writes the exact f32 -|k|^2 bias into the PSUM tile;
         all matmuls use start=False and accumulate onto it (warmup
         matmuls pre-set the has_written bits of the 4 rotating PSUM
         buffers once, so the PE accumulates instead of overwriting).
      2. 8x fp16 matmuls (K=128, N=512): fp16(x).T @ fp16(2k) hi*hi term.
         The PE multiplies fp16 operands exactly (fp22 internally) and
         accumulates in fp32.
      3. 8x fp8-e4m3 DoubleRow matmuls (K=256 packed, N=512) for the two
         cross terms: fp8(xl*64) . fp8(2k/64) and fp8(x/32) . fp8(kl*32),
         where xl = x - fp16(x), kl = 2k - fp16(2k); scales keep the
         operands inside e4m3 range. DoubleRow packs 2 contraction rows
         per PE cell, halving the matmul count for these passes.
    A matmul instruction costs ~N cycles regardless of dtype, so the tile
    costs 16 matmuls vs 24 for the bf16 hi/lo x3 scheme (1.5x less PE
    time); matmuls are grouped by PE mode (fp16, then DoubleRow) to
    minimize mode-switch stalls. Numpy-sim of this exact arithmetic
    flips 1 of 65536 argmaxes (rel err 5.5e-3, gate 2e-2).
  - DVE per tile: MAX8 + FIND_INDEX8 read straight from PSUM (no
    PSUM->SBUF move), then a gpsimd indirect-DMA gathers fp16 values
    rows and the result is written out from the Scalar queue; the host
    upcasts the fp16 output to f32 (values fp16 rounding adds ~1e-3 rel,
    far under the gate, and halves the gather+store HBM traffic).
"""

import numpy as np

_B = 65536
_D = 512
_K = 1024
_NCORES = 8
_BL = _B // _NCORES  # 8192 rows per core
_P = 128
_BBLK = 512          # b columns loaded per DMA
_BT = 128            # b rows per matmul tile (PSUM partition dim)
_DC = _D // _P       # 4 contraction chunks
_AX = 64.0           # scale for xl-cross fp8 pass
_AK = 32.0           # scale for kl-cross fp8 pass
_NPS = 4             # rotating score PSUM buffers (4 x 2 banks = all 8)

_cached = None


def _build():
    import concourse.mybir as mybir
    from concourse import bacc
    from concourse.bass import IndirectOffsetOnAxis
    from concourse.tile import TileContext

    f32 = mybir.dt.float32
    f16 = mybir.dt.float16
    f8 = mybir.dt.float8e4
    u32 = mybir.dt.uint32
    bf16 = mybir.dt.bfloat16
    DR = mybir.MatmulPerfMode.DoubleRow

    nc = bacc.Bacc("TRN2", target_bir_lowering=False, debug=False,
                   num_devices=_NCORES)
    xh16 = nc.dram_tensor("xh16", [_D, _BL], f16, kind="ExternalInput")
    xl8 = nc.dram_tensor("xl8", [_D, _BL], f8, kind="ExternalInput")
    xf8 = nc.dram_tensor("xf8", [_D, _BL], f8, kind="ExternalInput")
    k16 = nc.dram_tensor("k16", [_D, _K], f16, kind="ExternalInput")
    k8 = nc.dram_tensor("k8", [_D, _K], f8, kind="ExternalInput")
    kl8 = nc.dram_tensor("kl8", [_D, _K], f8, kind="ExternalInput")
    biasf = nc.dram_tensor("biasf", [1, _K], f32, kind="ExternalInput")
    vals = nc.dram_tensor("vals", [_K, _D], f16, kind="ExternalInput")
    out = nc.dram_tensor("out", [_BL, _D], f16, kind="ExternalOutput")

    xh3 = xh16.rearrange("(do p) b -> p do b", p=_P)    # [128, 4, 8192]
    xl4 = xl8.rearrange("(c j p) b -> p c j b", p=_P, c=2, j=2)
    xf4 = xf8.rearrange("(c j p) b -> p c j b", p=_P, c=2, j=2)
    k16_3 = k16.rearrange("(do p) k -> p do k", p=_P)   # [128, 4, 1024]
    k8_4 = k8.rearrange("(c j p) k -> p c j k", p=_P, c=2, j=2)
    kl8_4 = kl8.rearrange("(c j p) k -> p c j k", p=_P, c=2, j=2)

    with TileContext(nc) as tc:
        with (
            tc.tile_pool(name="const", bufs=1) as cpool,
            tc.tile_pool(name="xp", bufs=3) as xpool,
            tc.tile_pool(name="warm", bufs=1) as warmpool,
            tc.tile_pool(name="st", bufs=4) as stpool,
            tc.tile_pool(name="gp", bufs=4) as gpool,
            tc.tile_pool(name="ps", bufs=_NPS, space="PSUM") as pspool,
        ):
            # All input loads share the Sync engine's HWDGE queue so they
            # drain in consumption order (tile0's needs first); split queues
            # let the x-block prefetch flood starve the tiny const loads.
            bias_row = cpool.tile([1, _K], f32)
            bias_sb = cpool.tile([_P, _K], f32)
            k16_sb = cpool.tile([_P, _DC, _K], f16)
            k8_sb = cpool.tile([_P, 2, 2, _K], f8)
            kl8_sb = cpool.tile([_P, 2, 2, _K], f8)
            nc.sync.dma_start(bias_row[:], biasf[:, :])
            nc.sync.dma_start(k16_sb[:, :, 0:512], k16_3[:, :, 0:512])
            nc.sync.dma_start(k16_sb[:, :, 512:1024], k16_3[:, :, 512:1024])

            # x block0 (128 rows) loads right behind k16 so tile0 can start
            # the moment the PE warmup finishes; k8/kl8 are only consumed
            # from tile0's 9th matmul on, so they queue after it.
            xh_t0 = xpool.tile([_P, _DC, _BBLK], f16, tag="xh")
            xl_t0 = xpool.tile([_P, 2, 2, _BBLK], f8, tag="xl")
            xf_t0 = xpool.tile([_P, 2, 2, _BBLK], f8, tag="xf")
            nc.sync.dma_start(xh_t0[:, :, :_BT], xh3[:, :, 0:_BT])
            nc.sync.dma_start(xl_t0[:, :, :, :_BT], xl4[:, :, :, 0:_BT])
            nc.sync.dma_start(xf_t0[:, :, :, :_BT], xf4[:, :, :, 0:_BT])

            nc.sync.dma_start(k8_sb[:, :, :, 0:512], k8_4[:, :, :, 0:512])
            nc.sync.dma_start(kl8_sb[:, :, :, 0:512], kl8_4[:, :, :, 0:512])

            # Warmup operands memset on GpSimd (its engine comes up ~1.5us
            # before VectorE).
            ones = warmpool.tile([1, _P], f32)
            nc.gpsimd.memset(ones[:], 1.0)
            wlhs = warmpool.tile([_P, _P], bf16)
            nc.gpsimd.memset(wlhs[:], 0.0)
            wrhs = warmpool.tile([_P, 512], bf16)
            nc.gpsimd.memset(wrhs[:], 0.0)

            # Broadcast the 4KB bias row to all 128 partitions with a pair
            # of K=1 fp32 matmuls (ones.T @ bias_row, exact), then copy
            # PSUM -> SBUF for the later tiles. The two cold fp32 4-pass
            # matmuls double as the PE clock (HAM) warmup. Tile0 skips its
            # bias copy entirely: it accumulates straight onto this PSUM
            # tile, whose has_written bits the broadcast matmuls set.
            # The other rotating score PSUM buffers are pre-touched with
            # start=True zero matmuls for the same reason: steady-state
            # tiles never use start=True (the bias is pre-written by
            # ScalarE and matmuls accumulate onto it), and a PE write with
            # has_written=0 would overwrite the bias instead of
            # accumulating.
            for b in range(_NPS - 1):
                wtile = pspool.tile([_P, _K], f32, tag="ps")
                nc.tensor.matmul(wtile[:, 0:512], lhsT=wlhs[:],
                                 rhs=wrhs[:], start=True, stop=True)
                nc.tensor.matmul(wtile[:, 512:1024], lhsT=wlhs[:],
                                 rhs=wrhs[:], start=True, stop=True)
            btile = pspool.tile([_P, _K], f32, tag="ps")
            nc.tensor.matmul(btile[:, 0:512], lhsT=ones[:],
                             rhs=bias_row[:, 0:512], start=True, stop=True)
            nc.tensor.matmul(btile[:, 512:1024], lhsT=ones[:],
                             rhs=bias_row[:, 512:1024], start=True, stop=True)
            nc.scalar.copy(out=bias_sb[:], in_=btile[:])

            # Remaining const halves, queued behind tile0's critical loads
            # but ahead of the bulk x prefetch below.
            nc.sync.dma_start(k8_sb[:, :, :, 512:1024], k8_4[:, :, :, 512:1024])
            nc.sync.dma_start(kl8_sb[:, :, :, 512:1024], kl8_4[:, :, :, 512:1024])

            blocks = [(0, _BT)]
            off = _BT
            while off < _BL:
                w = min(_BBLK, _BL - off)
                blocks.append((off, w))
                off += w

            for bi, (boff, bw) in enumerate(blocks):
                if bi == 0:
                    xh_t, xl_t, xf_t = xh_t0, xl_t0, xf_t0
                else:
                    xh_t = xpool.tile([_P, _DC, _BBLK], f16, tag="xh")
                    xl_t = xpool.tile([_P, 2, 2, _BBLK], f8, tag="xl")
                    xf_t = xpool.tile([_P, 2, 2, _BBLK], f8, tag="xf")
                    nc.sync.dma_start(xh_t[:, :, :bw], xh3[:, :, boff:boff + bw])
                    nc.sync.dma_start(xl_t[:, :, :, :bw], xl4[:, :, :, boff:boff + bw])
                    nc.sync.dma_start(xf_t[:, :, :, :bw], xf4[:, :, :, boff:boff + bw])

                for sub in range(bw // _BT):
                    bt = boff // _BT + sub
                    bsl = slice(sub * _BT, (sub + 1) * _BT)
                    if bt == 0:
                        ps = btile
                    else:
                        ps = pspool.tile([_P, _K], f32, tag="ps")
                    hs = [slice(0, 512), slice(512, 1024)]
                    # ScalarE pre-writes the exact f32 -|k|^2 bias into PSUM;
                    # all matmuls then accumulate onto it (start=False; the
                    # has_written bits were set once by the warmup matmuls).
                    # Tile0 lands on the bias-broadcast PSUM buffer, which
                    # already holds the bias, so it skips the copy.
                    # Matmuls grouped by PE mode (fp16 then fp8-DR) to
                    # minimize mode-switch stalls; within the fp16 group,
                    # dc outer / h inner so consecutive matmuls share the
                    # same stationary operand.
                    if bt > 0:
                        nc.scalar.copy(out=ps[:], in_=bias_sb[:])
                    for dc in range(_DC):
                        for h in range(2):
                            nc.tensor.matmul(ps[:, hs[h]], lhsT=xh_t[:, dc, bsl],
                                             rhs=k16_sb[:, dc, hs[h]],
                                             start=False, stop=False,
                                             skip_group_check=True)
                    for c in range(2):
                        for h in range(2):
                            nc.tensor.matmul(ps[:, hs[h]], lhsT=xl_t[:, c, :, bsl],
                                             rhs=k8_sb[:, c, :, hs[h]],
                                             perf_mode=DR,
                                             start=False, stop=False,
                                             skip_group_check=True)
                    for c in range(2):
                        for h in range(2):
                            nc.tensor.matmul(ps[:, hs[h]], lhsT=xf_t[:, c, :, bsl],
                                             rhs=kl8_sb[:, c, :, hs[h]],
                                             perf_mode=DR,
                                             start=False, stop=(c == 1),
                                             skip_group_check=True)
                    mx = stpool.tile([_P, 8], f32)
                    nc.vector.max(out=mx[:], in_=ps[:])
                    idx = stpool.tile([_P, 8], u32)
                    nc.vector.max_index(out=idx[:], in_max=mx[:], in_values=ps[:])

                    g = gpool.tile([_P, _D], f16)
                    nc.gpsimd.indirect_dma_start(
                        out=g[:],
                        out_offset=None,
                        in_=vals[:, :],
                        in_offset=IndirectOffsetOnAxis(ap=idx[:, :1], axis=0),
                    )
                    nc.scalar.dma_start(out[bt * _BT:(bt + 1) * _BT, :], g[:])

    nc.compile()
    return nc


def _get_nc():
    global _cached
    if _cached is None:
        _cached = _build()
    return _cached


def _fp8(a):
    import ml_dtypes

    return np.clip(a, -240.0, 240.0).astype(ml_dtypes.float8_e4m3)


def _prepare_in_maps(x, keys, values):
    x = np.asarray(x, dtype=np.float32)
    keys = np.asarray(keys, dtype=np.float32)
    values = np.asarray(values, dtype=np.float32)

    kT = np.ascontiguousarray((2.0 * keys).T)            # [512, 1024] f32
    k16 = kT.astype(np.float16)
    kl = kT - k16.astype(np.float32)
    k8 = _fp8(kT / _AX)
    kl8 = _fp8(kl * _AK)

    k2 = np.einsum("kd,kd->k", keys.astype(np.float64),
                   keys.astype(np.float64))
    biasf = np.ascontiguousarray((-k2).astype(np.float32)[None, :])
    vals16 = values.astype(np.float16)

    in_maps = []
    for c in range(_NCORES):
        xs = np.ascontiguousarray(x[c * _BL:(c + 1) * _BL].T)  # [512, 8192]
        xh16 = xs.astype(np.float16)
        xl = xs - xh16.astype(np.float32)
        xl8 = _fp8(xl * _AX)
        xf8 = _fp8(xs / _AK)
        in_maps.append({
            "xh16": xh16, "xl8": xl8, "xf8": xf8,
            "k16": k16, "k8": k8, "kl8": kl8,
            "biasf": biasf, "vals": vals16,
        })
    return in_maps


def kernel(x, keys, values):
    from concourse.bass_utils import run_bass_kernel_spmd

    nc = _get_nc()
    in_maps = _prepare_in_maps(x, keys, values)
    res = run_bass_kernel_spmd(nc, in_maps, core_ids=list(range(_NCORES)))
    return np.concatenate([r["out"] for r in res.results],
                          axis=0).astype(np.float32)
